# revision 36
# baseline (speedup 1.0000x reference)
"""Trainium2 Bass kernel for Autoformer AutoCorrelation attention.

Reference computation (per batch b):
  q = (x_q @ Wq + bq), k, v likewise                               (L, D)
  corr[h,e,:] = irfft(rfft(q_che) * conj(rfft(k_che)))             circular cross-corr
  mean_value[b,:] = corr.mean(over H,E)                            (L,)
  index = top_k(mean_value.mean(over B), 7)                        global over batch
  w = softmax(mean_value[b, index])
  agg = sum_k w[k] * roll(v, -index[k], axis=time)
  out = agg @ Wo + bo

Key identities used (no FFT needed):
  * mean_value[b, tau] = (1/D) * sum_t <q_proj[t+tau], k_proj[t]>  = circular
    diagonal sums of the Gram matrix G = q_proj @ k_proj^T, computed as
    matmuls on the PE with a window trick so that each PSUM column holds a
    fixed (t' - t) difference; a log2(128)-step shifted partition fold then
    yields all 2048 diagonal sums.
  * roll commutes with the output projection: out = sum_k w[k] *
    roll(v_proj @ Wo + bo) since sum_k w[k] = 1. So the dynamic gather is 7
    offset reads of O^T = (v_proj @ Wo + bo)^T along the free dim.

Sharding: data-parallel over batch, one batch item per core (B == 8 cores).
The only cross-core exchange is an AllReduce of the (L,) mean stat.

All matmuls run in float32r (fp32 storage, ~1.5e-4 rel err, 4x faster than
fp32 on the PE).
"""

import functools
import os
import sys

sys.path.insert(0, "/opt/trn_rl_repo")

import numpy as np

import concourse.bass as bass
import concourse.mybir as mybir
import concourse.tile as tile
from concourse import bacc, bass_utils
from concourse.masks import make_identity

P = 128
B, L, D = 8, 2048, 1024
NCORES = 8
TOPK = int(os.environ.get("KERNEL_TOPK", "7"))
KC = D // P   # 8 contraction chunks
MT = D // P   # 8 output-row tiles
TT = L // P   # 16 time tiles
TC = 256      # time-chunk width for projections
NTCH = L // TC
GW = 512      # gram matmul moving width (one PSUM bank)

f32 = mybir.dt.float32
f32r = mybir.dt.float32r
u32 = mybir.dt.uint32
i32 = mybir.dt.int32
AF = mybir.ActivationFunctionType
ALU = mybir.AluOpType

DEBUG_OUTS = os.environ.get("KERNEL_DEBUG", "0") == "1"
SIM_MODE = os.environ.get("KERNEL_SIM", "0") == "1"   # single-core, no collective


def _emit_projection(nc, natp, xtp, psT, psP, ident, x_dram, w_sb, b_col, out_tiles,
                     copy_engine):
    """out_tiles[m][:, :] (dout tile m on partitions, t free) = (x @ W + b)^T."""
    for c in range(NTCH):
        xt = xtp.tile([P, KC, TC], f32r, tag="xt")
        for j in range(TC // P):
            rows = x_dram[(c * TC + j * P):(c * TC + (j + 1) * P), :]
            for h in range(2):
                nat = natp.tile([P, D // 2], f32, tag="nat")
                eng = nc.sync if h == 0 else nc.scalar
                eng.dma_start(nat[:], rows[:, h * (D // 2):(h + 1) * (D // 2)])
                for kh in range(KC // 2):
                    kc = h * (KC // 2) + kh
                    pst = psT.tile([P, P], f32, tag="psT")
                    nc.tensor.transpose(pst[:], nat[:, kh * P:(kh + 1) * P], ident)
                    if kc % 2 == 0:
                        nc.vector.tensor_copy(xt[:, kc, j * P:(j + 1) * P], pst[:])
                    else:
                        nc.scalar.activation(xt[:, kc, j * P:(j + 1) * P], pst[:],
                                             AF.Identity)
        for m in range(MT):
            psp = psP.tile([P, TC], f32, tag="psP")
            for kc in range(KC):
                nc.tensor.matmul(psp[:], w_sb[:, kc, m * P:(m + 1) * P],
                                 xt[:, kc, :], start=(kc == 0), stop=(kc == KC - 1))
            copy_engine(out_tiles[m][:, c * TC:(c + 1) * TC], psp[:], b_col[:, m:m + 1])


def _build():
    nc = bacc.Bacc("TRN2", target_bir_lowering=False, debug=False,
                   num_devices=1 if SIM_MODE else NCORES)

    x_q = nc.dram_tensor("queries", [L, D], f32, kind="ExternalInput").ap()
    x_k = nc.dram_tensor("keys", [L, D], f32, kind="ExternalInput").ap()
    x_v = nc.dram_tensor("values", [L, D], f32, kind="ExternalInput").ap()
    w_q = nc.dram_tensor("Wq", [D, D], f32r, kind="ExternalInput").ap()
    w_k = nc.dram_tensor("Wk", [D, D], f32r, kind="ExternalInput").ap()
    w_v = nc.dram_tensor("Wv", [D, D], f32r, kind="ExternalInput").ap()
    w_o = nc.dram_tensor("Wo", [D, D], f32r, kind="ExternalInput").ap()
    b_q = nc.dram_tensor("bq", [D], f32, kind="ExternalInput").ap()
    b_k = nc.dram_tensor("bk", [D], f32, kind="ExternalInput").ap()
    b_v = nc.dram_tensor("bv", [D], f32, kind="ExternalInput").ap()
    b_o = nc.dram_tensor("bo", [D], f32, kind="ExternalInput").ap()
    out = nc.dram_tensor("out", [L, D], f32, kind="ExternalOutput").ap()
    if DEBUG_OUTS:
        dbg_mv = nc.dram_tensor("dbg_mv", [1, L], f32, kind="ExternalOutput").ap()
        dbg_mvsum = nc.dram_tensor("dbg_mvsum", [1, L], f32, kind="ExternalOutput").ap()
        dbg_idx = nc.dram_tensor("dbg_idx", [1, 8], u32, kind="ExternalOutput").ap()
        dbg_w = nc.dram_tensor("dbg_w", [1, 8], f32, kind="ExternalOutput").ap()
        dbg_qt = nc.dram_tensor("dbg_qt", [P, L], f32r, kind="ExternalOutput").ap()
        dbg_psg_lo = nc.dram_tensor("dbg_psg_lo", [64, L], f32, kind="ExternalOutput").ap()
        dbg_psg_hi = nc.dram_tensor("dbg_psg_hi", [64, L], f32, kind="ExternalOutput").ap()
        dbg_f0 = nc.dram_tensor("dbg_f0", [64, L], f32, kind="ExternalOutput").ap()

    from contextlib import ExitStack

    with tile.TileContext(nc) as tc:
        with (
            tc.tile_pool(name="smalls", bufs=1) as smalls,
            tc.tile_pool(name="psT", bufs=2, space="PSUM") as psT,
            tc.tile_pool(name="psP", bufs=2, space="PSUM") as psP,
            tc.tile_pool(name="dram", bufs=1, space="DRAM") as dram,
        ):
            proj_es = ExitStack()
            natp = proj_es.enter_context(tc.tile_pool(name="natp", bufs=2, side="right"))
            xtp = proj_es.enter_context(tc.tile_pool(name="xtp", bufs=3, side="right"))
            wp = proj_es.enter_context(tc.tile_pool(name="wp", bufs=1, side="right"))
            ident_t = smalls.tile([P, P], f32, tag="ident")
            make_identity(nc, ident_t[:])
            ident = ident_t[:]

            # biases arranged (p, m): element = b[m*128 + p]
            bias_sb = {}
            for name, bdram in (("q", b_q), ("k", b_k), ("v", b_v), ("o", b_o)):
                t = smalls.tile([P, MT], f32, tag=f"bias_{name}")
                nc.sync.dma_start(t[:], bdram.rearrange("(m p) -> p m", p=P))
                bias_sb[name] = t

            def act_copy(dst, src, bias_ap):
                nc.scalar.activation(dst, src, AF.Identity, bias=bias_ap)

            # ---------------- phase A: q/k projections + gram + fold ----------
            with (
                tc.tile_pool(name="qkp", bufs=1) as qkp,
                tc.tile_pool(name="psG", bufs=1, space="PSUM") as psG,
                tc.tile_pool(name="foldp", bufs=1) as foldp,
            ):
                q_t = [qkp.tile([P, L], f32r, tag=f"qT{m}", name=f"qT{m}") for m in range(MT)]
                k_t = [qkp.tile([P, L], f32r, tag=f"kT{m}", name=f"kT{m}") for m in range(MT)]

                wq_sb = wp.tile([P, KC, D], f32r, tag="w")
                nc.gpsimd.dma_start(wq_sb[:], w_q.rearrange("(kc p) d -> p kc d", p=P))
                _emit_projection(nc, natp, xtp, psT, psP, ident, x_q, wq_sb,
                                 bias_sb["q"], q_t, act_copy)

                wk_sb = wp.tile([P, KC, D], f32r, tag="w")
                nc.gpsimd.dma_start(wk_sb[:], w_k.rearrange("(kc p) d -> p kc d", p=P))
                _emit_projection(nc, natp, xtp, psT, psP, ident, x_k, wk_sb,
                                 bias_sb["k"], k_t, act_copy)

                if DEBUG_OUTS:
                    nc.sync.dma_start(dbg_qt, q_t[0][:])

                # Gram: psg[r, q] accumulates G[128i + r, (128i + q) % L]
                # over i, mc  =>  tau = (r - q) mod L per element. Window
                # start 128i (not 128(i+1)) so that i == 0 (the start=True
                # pass) is exactly bank-aligned: a wrapped start=True pair
                # would reset the whole PSUM bank twice and lose the first
                # piece.
                psg = psG.tile([P, L], f32, tag="psG")
                for i in range(TT):
                    for mc in range(MT):
                        lhs = q_t[mc][:, i * P:(i + 1) * P]
                        st = (i == 0 and mc == 0)
                        sp = (i == TT - 1 and mc == MT - 1)
                        for g in range(L // GW):
                            s = (P * i + GW * g) % L
                            e = s + GW
                            if e <= L:
                                nc.tensor.matmul(psg[:, g * GW:(g + 1) * GW], lhs,
                                                 k_t[mc][:, s:e], start=st, stop=sp,
                                                 skip_group_check=True)
                            else:
                                a = L - s
                                nc.tensor.matmul(psg[:, g * GW:g * GW + a], lhs,
                                                 k_t[mc][:, s:L], start=st, stop=sp,
                                                 skip_group_check=True)
                                nc.tensor.matmul(psg[:, g * GW + a:(g + 1) * GW], lhs,
                                                 k_t[mc][:, 0:e - L], start=st, stop=sp,
                                                 skip_group_check=True)

                # fold: S[q] = sum_r psg[r, (q + r) % L]; 7 shifted halvings.
                # mv_own[q] = sum over dout of corr[b, tau], tau = (-q-128) % L
                # Each step: DMA rebases partitions [h:2h] to base 0 with the
                # circular +h column shift applied, then one in-place add.
                # (DVE two-SBUF-input ops require equal base partitions.)
                # First step: stage psg's upper half in SBUF at base
                # partition 0 (DVE copy rebases; DMA cannot read PSUM), then
                # add with the circular +64 shift via free-dim offsets.
                tmp64 = foldp.tile([64, L], f32, tag="foldA")
                nc.vector.tensor_copy(tmp64[:], psg[64:128, :])
                if DEBUG_OUTS:
                    psg_lo = foldp.tile([64, L], f32, tag="foldB")
                    nc.vector.tensor_copy(psg_lo[:], psg[0:64, :])
                    nc.sync.dma_start(dbg_psg_lo, psg_lo[:])
                    nc.sync.dma_start(dbg_psg_hi, tmp64[:])
                f0 = foldp.tile([64, L], f32, tag="foldB")
                nc.vector.tensor_add(f0[:, 0:L - 64], psg[0:64, 0:L - 64],
                                     tmp64[:, 64:L])
                nc.vector.tensor_add(f0[:, L - 64:L], psg[0:64, L - 64:L],
                                     tmp64[:, 0:64])
                if DEBUG_OUTS:
                    nc.sync.dma_start(dbg_f0, f0[:])
                src = f0
                tag_flip = True
                for h in (32, 16, 8, 4, 2, 1):
                    tmp = foldp.tile([h, L], f32, tag="foldA" if tag_flip else "foldB")
                    tag_flip = not tag_flip
                    nc.scalar.dma_start(tmp[:, 0:L - h], src[h:2 * h, h:L])
                    nc.scalar.dma_start(tmp[:, L - h:L], src[h:2 * h, 0:h])
                    nc.vector.tensor_add(tmp[:], src[0:h, :], tmp[:])
                    src = tmp
                mv_own = src  # (1, L) tile; h=1 lands on tag foldB

                # ---------------- collective: sum over batch ------------------
                ar_in = dram.tile([1, L], f32, tag="ar_in")
                ar_out = dram.tile([1, L], f32, tag="ar_out")
                nc.scalar.dma_start(ar_in[:], mv_own[:])
                if DEBUG_OUTS:
                    nc.sync.dma_start(dbg_mv, mv_own[:])

            # ---------------- phase B: v proj, O = (vWo + bo), agg, out -------
            if SIM_MODE:
                nc.sync.dma_start(ar_out[:], ar_in[:])
            else:
                nc.gpsimd.collective_compute(
                    "AllReduce", ALU.add,
                    replica_groups=[list(range(NCORES))],
                    ins=[ar_in[:].opt()], outs=[ar_out[:].opt()],
                )
            O_dram = dram.tile([L, D], f32, tag="O_dram")

            with tc.tile_pool(name="vtp", bufs=1) as vtp:
                v_t = [vtp.tile([P, L], f32r, tag=f"vT{m}", name=f"vT{m}") for m in range(MT)]
                wv_sb = wp.tile([P, KC, D], f32r, tag="w")
                nc.gpsimd.dma_start(wv_sb[:], w_v.rearrange("(kc p) d -> p kc d", p=P))
                _emit_projection(nc, natp, xtp, psT, psP, ident, x_v, wv_sb,
                                 bias_sb["v"], v_t, act_copy)

                proj_es.close()  # free natp/xtp/wp before the big phase-B pools
                with (
                    tc.tile_pool(name="wop", bufs=1) as wop,
                    tc.tile_pool(name="otp", bufs=3) as otp,
                    tc.tile_pool(name="smb", bufs=1) as smb,
                    tc.tile_pool(name="gthp", bufs=6) as gthp,
                    tc.tile_pool(name="up", bufs=3) as up,
                    tc.tile_pool(name="outp", bufs=6) as outp,
                    tc.tile_pool(name="psF", bufs=4, space="PSUM") as psF,
                ):
                    # ---- O = v_proj @ Wo + bo, transposed to (t, d), to DRAM
                    wo_sb = wop.tile([P, KC, D], f32r, tag="wo")
                    nc.sync.dma_start(wo_sb[:],
                                      w_o.rearrange("(kc p) d -> p kc d", p=P))
                    for m in range(MT):
                        ot = otp.tile([P, L], f32, tag="ot")
                        for c in range(NTCH):
                            pso = psP.tile([P, TC], f32, tag="psP")
                            for dc in range(KC):
                                nc.tensor.matmul(pso[:],
                                                 wo_sb[:, dc, m * P:(m + 1) * P],
                                                 v_t[dc][:, c * TC:(c + 1) * TC],
                                                 start=(dc == 0), stop=(dc == KC - 1))
                            nc.scalar.activation(ot[:, c * TC:(c + 1) * TC], pso[:],
                                                 AF.Identity,
                                                 bias=bias_sb["o"][:, m:m + 1])
                        for i in range(TT):
                            psf = psF.tile([P, P], f32, tag="psF")
                            nc.tensor.transpose(psf[:], ot[:, i * P:(i + 1) * P],
                                                ident)
                            osb = outp.tile([P, P], f32, tag="osb")
                            nc.vector.tensor_copy(osb[:], psf[:])
                            (nc.scalar if i % 2 == 0 else nc.sync).dma_start(
                                O_dram[i * P:(i + 1) * P, m * P:(m + 1) * P],
                                osb[:])

                    # ---- top-k over the batch-summed stat --------------------
                    mv_sum = smb.tile([1, L], f32, tag="mv_sum")
                    nc.sync.dma_start(mv_sum[:], ar_out[:])
                    top_vals = smb.tile([1, 8], f32, tag="top_vals")
                    top_idx = smb.tile([1, 8], u32, tag="top_idx")
                    nc.vector.max_with_indices(top_vals[:], top_idx[:], mv_sum[:])

                    # own-batch values at the top-k positions: gather from
                    # ar_in (this core's mv) with the indices as a per-
                    # partition column.
                    # SBUF free<->partition transposes must bounce via DRAM.
                    idx_bounce = dram.tile([1, 8], u32, tag="idx_bounce")
                    nc.sync.dma_start(idx_bounce[:], top_idx[:])
                    idx_col = smb.tile([8, 1], u32, tag="idx_col")
                    nc.sync.dma_start(idx_col[:],
                                      idx_bounce[:].rearrange("o k -> k o"))
                    wvals_col = smb.tile([8, 1], f32, tag="wvals_col")
                    nc.gpsimd.indirect_dma_start(
                        out=wvals_col[:], out_offset=None,
                        in_=ar_in[:].rearrange("o q -> (o q) ()"),
                        in_offset=bass.IndirectOffsetOnAxis(ap=idx_col[:, 0:1],
                                                            axis=0),
                    )
                    wv_bounce = dram.tile([1, 8], f32, tag="wv_bounce")
                    nc.sync.dma_start(wv_bounce[:].rearrange("o k -> k o"),
                                      wvals_col[:])
                    wvals = smb.tile([1, 8], f32, tag="wvals")
                    nc.sync.dma_start(wvals[:], wv_bounce[:])

                    # lag for mv position q is tau = (-q) % L; gather row
                    # indices per time-tile: idx[p, i] = (tau + 128*i + p) % L.
                    # Arithmetic tensor_scalar needs f32 scalars, so use
                    # tensor_tensor int ops with constant tiles instead.
                    c3968 = smb.tile([1, 8], i32, tag="c3968")
                    nc.gpsimd.memset(c3968[:], 2 * L)
                    taus_row = smb.tile([1, 8], i32, tag="taus_row")
                    nc.vector.tensor_tensor(taus_row[:], c3968[:],
                                            top_idx[:].bitcast(i32),
                                            ALU.subtract)
                    taus_bc = smb.tile([P, 8], i32, tag="taus_bc")
                    nc.gpsimd.partition_broadcast(taus_bc[:], taus_row[:])
                    iota2 = smb.tile([P, TT], i32, tag="iota2")
                    nc.gpsimd.iota(iota2[:], pattern=[[P, TT]], base=0,
                                   channel_multiplier=1)
                    c2047 = smb.tile([P, 1], i32, tag="c2047")
                    nc.gpsimd.memset(c2047[:], L - 1)
                    idx_k = []
                    for k2 in range(TOPK):
                        ik = smb.tile([P, TT], i32, tag=f"idx_k{k2}",
                                      name=f"idx_k{k2}")
                        nc.vector.tensor_tensor(
                            ik[:], taus_bc[:, k2:k2 + 1].to_broadcast((P, TT)),
                            iota2[:], ALU.add)
                        nc.vector.tensor_tensor(
                            ik[:], ik[:], c2047[:].to_broadcast((P, TT)),
                            ALU.bitwise_and)
                        idx_k.append(ik)

                    # ---- per-batch softmax weights ---------------------------
                    w7 = smb.tile([P, 8], f32, tag="w7")
                    nc.gpsimd.partition_broadcast(w7[:], wvals[:])
                    wmax = smb.tile([P, 1], f32, tag="wmax")
                    nc.vector.tensor_reduce(wmax[:], w7[:, 0:TOPK],
                                            mybir.AxisListType.X, ALU.max)
                    negmax = smb.tile([P, 1], f32, tag="negmax")
                    nc.scalar.mul(negmax[:], wmax[:], -1.0 / D)
                    wexp = smb.tile([P, 8], f32, tag="wexp")
                    nc.scalar.activation(wexp[:, 0:TOPK], w7[:, 0:TOPK], AF.Exp,
                                         bias=negmax[:], scale=1.0 / D)
                    wsum = smb.tile([P, 1], f32, tag="wsum")
                    nc.vector.tensor_reduce(wsum[:], wexp[:, 0:TOPK],
                                            mybir.AxisListType.X, ALU.add)
                    wrec = smb.tile([P, 1], f32, tag="wrec")
                    nc.vector.reciprocal(wrec[:], wsum[:])
                    wfin = smb.tile([P, 8], f32, tag="wfin")
                    nc.vector.tensor_scalar_mul(wfin[:, 0:TOPK], wexp[:, 0:TOPK],
                                                wrec[:])

                    if DEBUG_OUTS:
                        nc.sync.dma_start(dbg_mvsum, mv_sum[:])
                        nc.sync.dma_start(dbg_idx, top_idx[:])
                        nc.sync.dma_start(dbg_w, wfin[:1, :])

                    # ---- agg: u[t, :] = sum_k w_k * O[(t + tau_k) % L, :] ----
                    for i in range(TT):
                        u = up.tile([P, D], f32, tag="u")
                        eng = nc.vector
                        for k2 in range(TOPK):
                            gth = gthp.tile([P, D], f32, tag="gth")
                            nc.gpsimd.indirect_dma_start(
                                out=gth[:], out_offset=None,
                                in_=O_dram[:],
                                in_offset=bass.IndirectOffsetOnAxis(
                                    ap=idx_k[k2][:, i:i + 1], axis=0),
                            )
                            if k2 == 0:
                                eng.tensor_scalar_mul(u[:], gth[:],
                                                      wfin[:, 0:1])
                            else:
                                eng.scalar_tensor_tensor(
                                    u[:], gth[:], wfin[:, k2:k2 + 1], u[:],
                                    ALU.mult, ALU.add)
                        (nc.scalar if i % 2 == 0 else nc.sync).dma_start(
                            out[i * P:(i + 1) * P, :], u[:])

    nc.compile()
    return nc


@functools.lru_cache(maxsize=1)
def _get_nc():
    return _build()


def kernel(queries, keys, values, Wq, bq, Wk, bk, Wv, bv, Wo, bo):
    nc = _get_nc()
    shared = {
        "Wq": np.ascontiguousarray(np.asarray(Wq, dtype=np.float32)),
        "Wk": np.ascontiguousarray(np.asarray(Wk, dtype=np.float32)),
        "Wv": np.ascontiguousarray(np.asarray(Wv, dtype=np.float32)),
        "Wo": np.ascontiguousarray(np.asarray(Wo, dtype=np.float32)),
        "bq": np.ascontiguousarray(np.asarray(bq, dtype=np.float32)),
        "bk": np.ascontiguousarray(np.asarray(bk, dtype=np.float32)),
        "bv": np.ascontiguousarray(np.asarray(bv, dtype=np.float32)),
        "bo": np.ascontiguousarray(np.asarray(bo, dtype=np.float32)),
    }
    queries = np.asarray(queries, dtype=np.float32)
    keys = np.asarray(keys, dtype=np.float32)
    values = np.asarray(values, dtype=np.float32)
    in_maps = []
    for c in range(NCORES):
        m = dict(shared)
        m["queries"] = np.ascontiguousarray(queries[c])
        m["keys"] = np.ascontiguousarray(keys[c])
        m["values"] = np.ascontiguousarray(values[c])
        in_maps.append(m)
    trace = os.environ.get("KERNEL_TRACE", "0") == "1"
    res = bass_utils.run_bass_kernel_spmd(nc, in_maps, core_ids=list(range(NCORES)),
                                          trace=trace)
    kernel.last_results = res
    return np.stack([res.results[c]["out"] for c in range(NCORES)])
